# revision 49
# baseline (speedup 1.0000x reference)
"""Trainium2 Bass kernel: fused causal MHA (qkv proj + causal softmax attn),
B=2, T=4096, C=768, nH=12, hd=64.

Sharding: 8 cores; core c -> batch b=c//4, head group g=c%4 (3 heads/core).

Per-core design (all matmuls bf16 operands, fp32 PSUM):
  Q^T/K^T proj: full cd=128 matmuls -> m-tiles [Q0;Q1],[K0;K1],[Q2;K2].
  V proj DIRECTLY in [key,dim] layout: out[t,d] = xT_chunk^T @ w_v chunk
    (lhsT = xT chunk, rhs = w_v) -- no PE transposes needed.
  S^T[k,q] = K Q^T as cd=64 matmuls on PE row-half tiles (lo=rows 0:63,
    hi=64:127). MMs on disjoint row groups CO-EXECUTE only when adjacent
    in program order, so the three per-round units are driven per-MM:
    h0 always lo, h1 always hi, h2 alternates sides per exp-group (its
    Q/K live in both halves via the swapped dup2 tile) -> each side gets
    ~1.5 units/round and S^T runs at ~2x column rate.
  Causal trim: for q-chunk J, diagonal kc blocks use rhs width 512-128d;
    leading [128,128] triangle of exp'd P^T zeroed by bf16 mask multiply.
  exp split across two engines: ACT (table exp, scale=1/8) and DVE
    (Schraudolph: pT_bf16 = bitcast(int16(A*S + B)), one tensor_scalar
    mult+add pass with f32->i16 round-to-nearest convert; ~3% per-element
    which the softmax normalization mostly averages out).
  PV: O^T_aug[65,512] += V_aug^T P^T (cd=128, ones col gives denom row).
  Output: unnormalized [O^T;denom] rows psum->sbuf->DRAM; HOST divides by
    denominator, transposes, and adds the V bias (out = num/den + b_v).
  Q/K bias: ACT Identity-with-bias on the psum->sbuf copy (per-partition).
  Schedule: proj chunk n is woven between attention units of round n-1.
PSUM budget: ps_lo 2 + po_h0 1 + ps_hi 2 + po_h1 1 + po_h2 1 + pj 1 = 8.
"""
import sys
sys.path.insert(0, '/opt/trn_rl_repo')
import numpy as np

import concourse.bass as bass
import concourse.tile as tile
from concourse import bacc, mybir
from concourse import bass_utils

B, T, C, NH = 2, 4096, 768, 12
HD = 64
HPC = 3
NCORES = 8
NQ = T // 512   # 8 q-chunks of 512
NKC = T // 128  # 32 key blocks of 128
VST = 208       # v_sb per-kc block stride (3*65=195 used, pad to 208)
GRP = 2         # S^T psum-bank group width (both sides)

# Schraudolph exp: bf16 = bitcast(int16(round(EXP_A*S + EXP_B)))
EXP_A = 184.6650558 * 0.125
EXP_B = 16248.6
ACT_SCALE = 0.125
# exp-group engine assignment: group ctr % DVE_PERIOD in DVE_SLOTS -> DVE
DVE_PERIOD = 7
DVE_SLOTS = (0, 2, 4)

BF = mybir.dt.bfloat16
F32 = mybir.dt.float32
F8 = mybir.dt.float8e4
I16 = mybir.dt.int16
AF = mybir.ActivationFunctionType
AL = mybir.AluOpType
DR = mybir.MatmulPerfMode.DoubleRow

_CACHE = {}


def _build():
    if 'nc' in _CACHE:
        return _CACHE['nc']
    nc = bacc.Bacc("TRN2", target_bir_lowering=False, debug=False,
                   enable_asserts=True, num_devices=NCORES)
    # host-packed layouts: one contiguous DMA per load
    #   xr[p, 3072n+512k+c] = x[b][512n+c, 128k+p]
    #   wqk[p, 384k+j]      = w_qk_stack[128k+p, j]   (j = 128m+c2)
    #   wv[p, 192k+j]       = w_v_stack[128k+p, j]
    xr_d = nc.dram_tensor("xr", [128, NQ * 3072], BF, kind="ExternalInput").ap()
    wqk_d = nc.dram_tensor("wqk", [128, 2304], BF, kind="ExternalInput").ap()
    wv_d = nc.dram_tensor("wv", [128, 1152], BF, kind="ExternalInput").ap()
    bqk_d = nc.dram_tensor("bqk", [128, 3], F32, kind="ExternalInput").ap()
    out_d = nc.dram_tensor("out", [HPC * 65, T], F32, kind="ExternalOutput").ap()

    dve_ctr = [0]

    with tile.TileContext(nc) as tc:
        with (
            tc.tile_pool(name="const", bufs=1) as cpool,
            tc.tile_pool(name="persist", bufs=1) as sb,
            tc.tile_pool(name="xn", bufs=4) as xpool,
            tc.tile_pool(name="pT", bufs=1) as ptp,
            tc.tile_pool(name="pj", bufs=1, space="PSUM") as pjp,
            tc.tile_pool(name="ps_lo", bufs=1, space="PSUM") as pslo,
            tc.tile_pool(name="ps_hi", bufs=1, space="PSUM") as pshi,
        ):
            # ---------- input DMAs first (hide latency) ----------
            xn = {}  # n -> [128, 3072] tile, chunk k at cols 512k:512(k+1)

            def load_xn(n):
                t = xpool.tile([128, 3072], BF, tag="xn", name=f"xn{n}")
                nc.sync.dma_start(t[:], xr_d[:, 3072 * n:3072 * (n + 1)])
                xn[n] = t

            t0 = xpool.tile([128, 3072], BF, tag="xn", name="xn0")
            wqk_all = sb.tile([128, 2304], BF, name="wqk")
            for k in range(6):
                nc.sync.dma_start(t0[:, 512 * k:512 * (k + 1)],
                                  xr_d[:, 512 * k:512 * (k + 1)],
                                  single_packet=True)
                nc.sync.dma_start(wqk_all[:, 384 * k:384 * (k + 1)],
                                  wqk_d[:, 384 * k:384 * (k + 1)],
                                  single_packet=True)
            xn[0] = t0
            wv_all = sb.tile([128, 1152], BF, name="wv")
            nc.sync.dma_start(wv_all[:], wv_d[:])
            load_xn(1)
            bias_qk = cpool.tile([128, 3], F32)
            nc.sync.dma_start(bias_qk[:], bqk_d[:])

            # ---------- constants ----------
            mask_tri = cpool.tile([128, 128], BF)
            nc.gpsimd.memset(mask_tri[:], 1.0)
            nc.gpsimd.affine_select(
                out=mask_tri[:], in_=mask_tri[:], compare_op=AL.is_ge,
                fill=0.0, base=0, channel_multiplier=-1, pattern=[[1, 128]])

            # Q/K tiles per n: q01=[Q0;Q1] k01=[K0;K1] qk2=[Q2;K2]
            # dup2=[K2;Q2] (halves swapped, via sbuf DMA)
            q01 = [sb.tile([128, 512], BF, name=f"q01_{n}") for n in range(NQ)]
            k01 = [sb.tile([128, 512], BF, name=f"k01_{n}") for n in range(NQ)]
            qk2 = [sb.tile([128, 512], BF, name=f"qk2_{n}") for n in range(NQ)]
            dup2 = [sb.tile([128, 512], BF, name=f"dup2_{n}") for n in range(NQ)]
            qs01 = [sb.tile([128, 512], BF, name=f"qs01_{n}") for n in range(NQ)]
            ks01 = [sb.tile([128, 512], BF, name=f"ks01_{n}") for n in range(NQ)]
            mdst = [q01, k01, qk2]
            # V storage: per kc block of 128 keys: [65 h0][65 h1][65 h2][pad]
            # with col 65h+64 = 1.0 (softmax denominator via ones column).
            v_sb = sb.tile([128, NKC * VST], BF, name="v_sb")
            # only the ones-columns need init (data cols written by proj,
            # pad cols never read): strided memset over cols VST*kc+65h+64
            v_r = v_sb[:].rearrange('p (a b) -> p a b', b=VST)
            nc.vector.memset(v_r[:, :, 64:195:65], 1.0)

            # ---------- work generators ----------
            def gen_proj(n, borrow=False):
                """Projection for t-chunk n: 3 QK m-tiles + 4 V t-subchunks.
                borrow=True (prologue only): rotate the psum accumulator
                across pj AND the still-idle po banks (pohi/po2) to
                triple-buffer the early projection chain -- no attention
                unit touches those banks until round 1."""
                if n + 2 < NQ:
                    load_xn(n + 2)
                tctr = [0]

                def pj_tile(nm):
                    sel = tctr[0] % 3 if borrow else 0
                    tctr[0] += 1
                    if sel == 1:
                        return pshi.tile([128, 512], F32, tag='po2',
                                         name=nm, bufs=1)
                    if sel == 2:
                        return pshi.tile([128, 512], F32, tag='pohi',
                                         name=nm, bufs=1)
                    return pjp.tile([128, 512], F32, tag="pj", name=nm,
                                    bufs=1)

                for m in range(3):
                    pj = pj_tile(f"pj{n}_{m}")
                    for k in range(6):
                        nc.tensor.matmul(
                            pj[:],
                            lhsT=wqk_all[:, 384 * k + 128 * m:
                                         384 * k + 128 * (m + 1)],
                            rhs=xn[n][:, 512 * k:512 * (k + 1)],
                            start=(k == 0), stop=(k == 5))
                    yield
                    # psum->sbuf copy + per-partition bias on ACT
                    nc.scalar.activation(
                        out=mdst[m][n][:], in_=pj[:], func=AF.Identity,
                        bias=bias_qk[:, m:m + 1], scale=1.0)
                # duplicate Q/K with halves swapped so every head's Q/K is
                # available on BOTH PE row-halves (enables deterministic
                # intra-unit lo/hi matmul pairing); issued from the idle
                # GPSIMD queue to keep the sync queue free for I/O DMAs
                nc.gpsimd.dma_start(dup2[n][0:64, :], qk2[n][64:128, :])
                nc.gpsimd.dma_start(dup2[n][64:128, :], qk2[n][0:64, :])
                nc.gpsimd.dma_start(qs01[n][0:64, :], q01[n][64:128, :])
                nc.gpsimd.dma_start(qs01[n][64:128, :], q01[n][0:64, :])
                nc.gpsimd.dma_start(ks01[n][0:64, :], k01[n][64:128, :])
                nc.gpsimd.dma_start(ks01[n][64:128, :], k01[n][0:64, :])
                for ti in range(4):
                    kc = 4 * n + ti
                    vp = pj_tile(f"vp{kc}")
                    for k in range(6):
                        nc.tensor.matmul(
                            vp[:, 0:192],
                            lhsT=xn[n][:, 512 * k + 128 * ti:
                                       512 * k + 128 * (ti + 1)],
                            rhs=wv_all[:, 192 * k:192 * (k + 1)],
                            start=(k == 0), stop=(k == 5))
                    yield
                    # single strided copy psum->v_sb (V bias added on host)
                    dst = v_sb[:, VST * kc:VST * kc + 195].rearrange(
                        'p (h c) -> p h c', c=65)[:, :, 0:64]
                    nc.vector.tensor_copy(
                        dst, vp[:, 0:192].rearrange('p (h c) -> p h c', c=64))

            def unit(h, J):
                """One (head, q-chunk) attention unit. Each group's two S^T
                matmuls use Q/K operands on OPPOSITE PE row-halves and are
                emitted back-to-back with no yield between, so they share
                identical readiness and deterministically co-execute (~2x).
                They write different PSUM banks (one 512-col window each)."""
                def kq(side):
                    if h == 0:
                        return (k01, q01) if side == 'lo' else (ks01, qs01)
                    if h == 1:
                        return (ks01, qs01) if side == 'lo' else (k01, q01)
                    return (dup2, qk2) if side == 'lo' else (qk2, dup2)
                popool, potag = {0: (pslo, 'polo'), 1: (pshi, 'pohi'),
                                 2: (pshi, 'po2')}[h]
                po = popool.tile([65, 512], F32, tag=potag,
                                 name=f"po_{h}_{J}", bufs=1)
                # kc groups: full chunks over [0, 4J), then trimmed diagonal
                # blocks packed into 512-col PSUM-bank windows (a matmul
                # output must never cross a 2KB PSUM bank boundary)
                groups = []
                g0 = 0
                while g0 < 4 * J:
                    g1 = min(g0 + GRP, 4 * J)
                    groups.append(([(kc, 512 * (kc - g0), 512)
                                    for kc in range(g0, g1)], False))
                    g0 = g1
                blocks = [(4 * J + d, 512 - 128 * d) for d in range(4)]
                cur, fill = [], [0] * GRP
                for kc, w in blocks:
                    wi = next((i for i in range(GRP)
                               if 512 - fill[i] >= w), None)
                    if wi is None:
                        groups.append((cur, True))
                        cur, fill = [], [0] * GRP
                        wi = 0
                    cur.append((kc, 512 * wi + fill[wi], w))
                    fill[wi] += w
                groups.append((cur, True))
                pending = None

                def gen_pv(pend):
                    blks, diag, ppT, last = pend
                    for i, (kc, off, w) in enumerate(blks):
                        d = kc - 4 * J
                        qc0 = 128 * d if diag else 0
                        nc.tensor.matmul(
                            po[:, qc0:512],
                            lhsT=v_sb[:, VST * kc + 65 * h:
                                      VST * kc + 65 * h + 65],
                            rhs=ppT[:, off:off + w],
                            start=(kc == 0), stop=(last and i == len(blks) - 1),
                            skip_group_check=True)
                        yield

                for gi, (blks, diag) in enumerate(groups):
                    # ps/pT buffer sharing: h0 on ps_lo, h1 on ps_hi, h2
                    # alternates (WAR-serialized against h0/h1 alternately)
                    pspool, pst = ((pslo, 'pslo') if h == 0 else
                                   (pshi, 'pshi') if h == 1 else
                                   ((pslo, 'pslo') if gi % 2 == 0
                                    else (pshi, 'pshi')))
                    wid = max(off + w for _, off, w in blks)
                    ps = pspool.tile([128, 512 * GRP], F32, tag=pst,
                                     name=f"ps_{h}_{J}_{gi}", bufs=1)
                    for bi, (kc, off, w) in enumerate(blks):
                        # side by PSUM-bank window: co-executing pair members
                        # must write DIFFERENT banks; same-window blocks stay
                        # same-side so they serialize instead of colliding
                        side = 'lo' if (off // 512) % 2 == 0 else 'hi'
                        hof = 0 if side == 'lo' else 64
                        ktile, qtile = kq(side)
                        qc0 = 128 * (kc - 4 * J) if diag else 0
                        nc.tensor.matmul(
                            ps[:, off:off + w],
                            lhsT=ktile[kc // 4][hof:hof + 64,
                                               128 * (kc % 4):
                                               128 * (kc % 4 + 1)],
                            rhs=qtile[J][hof:hof + 64, qc0:512],
                            start=True, stop=True)
                    yield
                    pT = ptp.tile([128, 512 * GRP], BF, tag=pst,
                                  name=f"pT_{h}_{J}_{gi}", bufs=4)
                    # per-unit engine: h0 exps on ACT, h1 on DVE, h2
                    # alternates -- each unit's ps-WAR chain is paced by ONE
                    # engine queue, decoupling the units' stall phases
                    use_dve = (h == 1) if h != 2 else (gi % 2 == 1)
                    if use_dve:
                        nc.vector.tensor_scalar(
                            out=pT[:, :wid].bitcast(I16), in0=ps[:, :wid],
                            scalar1=EXP_A, scalar2=EXP_B,
                            op0=AL.mult, op1=AL.add)
                    else:
                        nc.scalar.activation(pT[:, :wid], ps[:, :wid],
                                             AF.Exp, scale=ACT_SCALE)
                    dve_ctr[0] += 1
                    if diag:
                        # zero leading [128,128] triangle (q_loc < k) of each
                        # diagonal block: bf16 mask multiply in DVE 2x mode
                        for kc, off, w in blks:
                            nc.vector.tensor_tensor(
                                out=pT[:, off:off + 128],
                                in0=pT[:, off:off + 128],
                                in1=mask_tri[:], op=AL.mult)
                    if pending is not None:
                        yield from gen_pv(pending)
                    pending = (blks, diag, pT, gi == len(groups) - 1)
                    yield
                yield from gen_pv(pending)
                yield
                po_sb = ptp.tile([65, 512], F32, tag=f"posb{h}",
                                 name=f"posb_{h}_{J}", bufs=3)
                # psum->sbuf staging on ACT (DVE queue is the busier one)
                nc.scalar.copy(po_sb[:], po[:])
                nc.sync.dma_start(
                    out_d[65 * h:65 * (h + 1), 512 * J:512 * (J + 1)], po_sb[:])

            # ---------- weave: proj(r) among attention units of J=r-1 ------
            def drive(gens, slow=(), cadence=3):
                # gens in `slow` advance every cadence-th cycle so their
                # filler work spreads across the whole round
                alive = [True] * len(gens)
                cyc = 0
                while any(alive):
                    for i, g in enumerate(gens):
                        if alive[i] and (i not in slow or cyc % cadence == 0
                                         or not any(alive[j] for j in
                                                    range(len(gens))
                                                    if j not in slow)):
                            try:
                                next(g)
                            except StopIteration:
                                alive[i] = False
                    cyc += 1

            # prologue: interleave proj(0) and proj(1) so proj(1)'s matmuls
            # fill proj(0)'s copy/DMA tail before round 1 starts
            drive([gen_proj(0, borrow=True), gen_proj(1, borrow=True)])
            for r in range(1, NQ - 1):
                J = r - 1
                gens = [gen_proj(r + 1)]
                u0, u1, u2 = unit(0, J), unit(1, J), unit(2, J)
                # de-phase the units so their exp-WAR stalls on the PE
                # FIFO don't synchronize: advance h1 by 1 and h2 by 2
                # yields before the round-robin weave starts
                next(u1)
                next(u1)
                next(u2)
                next(u2)
                next(u2)
                next(u2)
                gens += [u0, u1, u2]
                # spread the 7 proj bursts across the round
                cad = max(3, (14 * J + 20) // 8)
                drive(gens, slow=(0,), cadence=cad)

            # final two rounds (J=NQ-2, NQ-1) have no proj: chain them per
            # head in ONE drive so the inter-round barrier disappears and
            # each head's tail drain overlaps the other heads' work
            def chain_units(h):
                for J in range(NQ - 2, NQ):
                    yield from unit(h, J)

            u0, u1, u2 = chain_units(0), chain_units(1), chain_units(2)
            next(u1)
            next(u1)
            next(u2)
            next(u2)
            next(u2)
            next(u2)
            drive([u0, u1, u2])

    nc.compile()
    _CACHE['nc'] = nc
    return nc


def _prep_inputs(x, w_qkv, b_qkv):
    """Host-side sharding: per-core packed x + reordered weight stacks.

    xr[p, 3072n+512k+c] = x[b][512n+c, 128k+p]  (one contiguous DMA per n)
    wqk[p, 384k+j] = w_qk_stack[128k+p, j]; wv[p, 192k+j] = w_v_stack[...]
    """
    import ml_dtypes
    cdt = ml_dtypes.bfloat16
    f8dt = ml_dtypes.float8_e4m3fn
    x = np.asarray(x, dtype=np.float32)
    w_qkv = np.asarray(w_qkv, dtype=np.float32)
    b_qkv = np.asarray(b_qkv, dtype=np.float32)
    xrs = []
    for b in range(B):
        xT = x[b].T.astype(cdt)  # [C, T]
        xr = np.ascontiguousarray(
            xT.reshape(6, 128, NQ, 512).transpose(1, 2, 0, 3).reshape(
                128, NQ * 3072))
        xrs.append(xr)
    in_maps = []
    for c in range(NCORES):
        b_idx, g = c // 4, c % 4
        H = [3 * g, 3 * g + 1, 3 * g + 2]
        q = lambda h: np.arange(64 * h, 64 * (h + 1))
        k = lambda h: np.arange(C + 64 * h, C + 64 * (h + 1))
        v = lambda h: np.arange(2 * C + 64 * h, 2 * C + 64 * (h + 1))
        qk_cols = np.concatenate([
            q(H[0]), q(H[1]), k(H[0]), k(H[1]), q(H[2]), k(H[2])])
        v_cols = np.concatenate([v(H[0]), v(H[1]), v(H[2])])
        wqk = w_qkv[:, qk_cols].astype(cdt).reshape(
            6, 128, 384).transpose(1, 0, 2).reshape(128, 2304)
        wv = w_qkv[:, v_cols].astype(cdt).reshape(
            6, 128, 192).transpose(1, 0, 2).reshape(128, 1152)
        bqk = np.zeros((128, 3), dtype=np.float32)
        for m in range(3):
            bqk[:, m] = b_qkv[qk_cols[128 * m:128 * (m + 1)]]
        in_maps.append({"xr": xrs[b_idx],
                        "wqk": np.ascontiguousarray(wqk),
                        "wv": np.ascontiguousarray(wv),
                        "bqk": bqk})
    return in_maps


def _run(x, w_qkv, b_qkv, n_head, **run_kwargs):
    assert int(n_head) == NH and x.shape == (B, T, C)
    nc = _build()
    in_maps = _prep_inputs(x, w_qkv, b_qkv)
    res = bass_utils.run_bass_kernel_spmd(
        nc, in_maps, core_ids=list(range(NCORES)), **run_kwargs)
    b_qkv = np.asarray(b_qkv, dtype=np.float32)
    out = np.empty((B, T, C), dtype=np.float32)
    for c in range(NCORES):
        b_idx, g = c // 4, c % 4
        o = res.results[c]["out"]  # [195, T]
        for h in range(HPC):
            ot = o[65 * h:65 * h + 64, :]       # unnormalized O^T
            den = o[65 * h + 64:65 * h + 65, :]  # softmax denominator
            bv = b_qkv[2 * C + 192 * g + 64 * h:2 * C + 192 * g + 64 * (h + 1)]
            out[b_idx, :, 192 * g + 64 * h:192 * g + 64 * (h + 1)] = (
                (ot / den).T + bv[None, :])
    return out, res


def kernel(x, w_qkv, b_qkv, n_head):
    return _run(x, w_qkv, b_qkv, n_head)[0]


# revision 51
# speedup vs baseline: 1.0217x; 1.0217x over previous
"""Trainium2 Bass kernel: fused causal MHA (qkv proj + causal softmax attn),
B=2, T=4096, C=768, nH=12, hd=64.

Sharding: 8 cores; core c -> batch b=c//4, head group g=c%4 (3 heads/core).

Per-core design (all matmuls bf16 operands, fp32 PSUM):
  Q^T/K^T proj: full cd=128 matmuls -> m-tiles [Q0;Q1],[K0;K1],[Q2;K2].
  V proj DIRECTLY in [key,dim] layout: out[t,d] = xT_chunk^T @ w_v chunk
    (lhsT = xT chunk, rhs = w_v) -- no PE transposes needed.
  S^T[k,q] = K Q^T as cd=64 matmuls on PE row-half tiles (lo=rows 0:63,
    hi=64:127). MMs on disjoint row groups CO-EXECUTE only when adjacent
    in program order, so the three per-round units are driven per-MM:
    h0 always lo, h1 always hi, h2 alternates sides per exp-group (its
    Q/K live in both halves via the swapped dup2 tile) -> each side gets
    ~1.5 units/round and S^T runs at ~2x column rate.
  Causal trim: for q-chunk J, diagonal kc blocks use rhs width 512-128d;
    leading [128,128] triangle of exp'd P^T zeroed by bf16 mask multiply.
  exp split across two engines: ACT (table exp, scale=1/8) and DVE
    (Schraudolph: pT_bf16 = bitcast(int16(A*S + B)), one tensor_scalar
    mult+add pass with f32->i16 round-to-nearest convert; ~3% per-element
    which the softmax normalization mostly averages out).
  PV: O^T_aug[65,512] += V_aug^T P^T (cd=128, ones col gives denom row).
  Output: unnormalized [O^T;denom] rows psum->sbuf->DRAM; HOST divides by
    denominator, transposes, and adds the V bias (out = num/den + b_v).
  Q/K bias: ACT Identity-with-bias on the psum->sbuf copy (per-partition).
  Schedule: proj chunk n is woven between attention units of round n-1.
PSUM budget: ps_lo 2 + po_h0 1 + ps_hi 2 + po_h1 1 + po_h2 1 + pj 1 = 8.
"""
import sys
sys.path.insert(0, '/opt/trn_rl_repo')
import numpy as np

import concourse.bass as bass
import concourse.tile as tile
from concourse import bacc, mybir
from concourse import bass_utils

B, T, C, NH = 2, 4096, 768, 12
HD = 64
HPC = 3
NCORES = 8
NQ = T // 512   # 8 q-chunks of 512
NKC = T // 128  # 32 key blocks of 128
VST = 208       # v_sb per-kc block stride (3*65=195 used, pad to 208)
GRP = 2         # S^T psum-bank group width (both sides)

# Schraudolph exp: bf16 = bitcast(int16(round(EXP_A*S + EXP_B)))
EXP_A = 184.6650558 * 0.125
EXP_B = 16248.6
ACT_SCALE = 0.125
# exp-group engine assignment: group ctr % DVE_PERIOD in DVE_SLOTS -> DVE
DVE_PERIOD = 7
DVE_SLOTS = (0, 2, 4)

BF = mybir.dt.bfloat16
F32 = mybir.dt.float32
F8 = mybir.dt.float8e4
I16 = mybir.dt.int16
AF = mybir.ActivationFunctionType
AL = mybir.AluOpType
DR = mybir.MatmulPerfMode.DoubleRow

_CACHE = {}


def _build():
    if 'nc' in _CACHE:
        return _CACHE['nc']
    nc = bacc.Bacc("TRN2", target_bir_lowering=False, debug=False,
                   enable_asserts=True, num_devices=NCORES)
    # host-packed layouts: one contiguous DMA per load
    #   xr[p, 3072n+512k+c] = x[b][512n+c, 128k+p]
    #   wqk[p, 384k+j]      = w_qk_stack[128k+p, j]   (j = 128m+c2)
    #   wv[p, 192k+j]       = w_v_stack[128k+p, j]
    xr_d = nc.dram_tensor("xr", [128, NQ * 3072], BF, kind="ExternalInput").ap()
    wqk_d = nc.dram_tensor("wqk", [128, 2304], BF, kind="ExternalInput").ap()
    wv_d = nc.dram_tensor("wv", [128, 1152], BF, kind="ExternalInput").ap()
    bqk_d = nc.dram_tensor("bqk", [128, 3], F32, kind="ExternalInput").ap()
    out_d = nc.dram_tensor("out", [HPC * 65, T], F32, kind="ExternalOutput").ap()

    dve_ctr = [0]

    with tile.TileContext(nc) as tc:
        with (
            tc.tile_pool(name="const", bufs=1) as cpool,
            tc.tile_pool(name="persist", bufs=1) as sb,
            tc.tile_pool(name="xn", bufs=4) as xpool,
            tc.tile_pool(name="pT", bufs=1) as ptp,
            tc.tile_pool(name="pj", bufs=1, space="PSUM") as pjp,
            tc.tile_pool(name="ps_lo", bufs=1, space="PSUM") as pslo,
            tc.tile_pool(name="ps_hi", bufs=1, space="PSUM") as pshi,
        ):
            # ---------- input DMAs first (hide latency) ----------
            xn = {}  # n -> [128, 3072] tile, chunk k at cols 512k:512(k+1)

            def load_xn(n):
                t = xpool.tile([128, 3072], BF, tag="xn", name=f"xn{n}")
                nc.sync.dma_start(t[:], xr_d[:, 3072 * n:3072 * (n + 1)])
                xn[n] = t

            t0 = xpool.tile([128, 3072], BF, tag="xn", name="xn0")
            wqk_all = sb.tile([128, 2304], BF, name="wqk")
            for k in range(6):
                nc.sync.dma_start(t0[:, 512 * k:512 * (k + 1)],
                                  xr_d[:, 512 * k:512 * (k + 1)],
                                  single_packet=True)
                nc.gpsimd.dma_start(wqk_all[:, 384 * k:384 * (k + 1)],
                                    wqk_d[:, 384 * k:384 * (k + 1)],
                                    single_packet=True)
            xn[0] = t0
            wv_all = sb.tile([128, 1152], BF, name="wv")
            nc.sync.dma_start(wv_all[:], wv_d[:])
            load_xn(1)
            bias_qk = cpool.tile([128, 3], F32)
            nc.sync.dma_start(bias_qk[:], bqk_d[:])

            # ---------- constants ----------
            mask_tri = cpool.tile([128, 128], BF)
            nc.gpsimd.memset(mask_tri[:], 1.0)
            nc.gpsimd.affine_select(
                out=mask_tri[:], in_=mask_tri[:], compare_op=AL.is_ge,
                fill=0.0, base=0, channel_multiplier=-1, pattern=[[1, 128]])

            # Q/K tiles per n: q01=[Q0;Q1] k01=[K0;K1] qk2=[Q2;K2]
            # dup2=[K2;Q2] (halves swapped, via sbuf DMA)
            q01 = [sb.tile([128, 512], BF, name=f"q01_{n}") for n in range(NQ)]
            k01 = [sb.tile([128, 512], BF, name=f"k01_{n}") for n in range(NQ)]
            qk2 = [sb.tile([128, 512], BF, name=f"qk2_{n}") for n in range(NQ)]
            dup2 = [sb.tile([128, 512], BF, name=f"dup2_{n}") for n in range(NQ)]
            qs01 = [sb.tile([128, 512], BF, name=f"qs01_{n}") for n in range(NQ)]
            ks01 = [sb.tile([128, 512], BF, name=f"ks01_{n}") for n in range(NQ)]
            mdst = [q01, k01, qk2]
            # V storage: per kc block of 128 keys: [65 h0][65 h1][65 h2][pad]
            # with col 65h+64 = 1.0 (softmax denominator via ones column).
            v_sb = sb.tile([128, NKC * VST], BF, name="v_sb")
            # only the ones-columns need init (data cols written by proj,
            # pad cols never read): strided memset over cols VST*kc+65h+64
            v_r = v_sb[:].rearrange('p (a b) -> p a b', b=VST)
            nc.vector.memset(v_r[:, :, 64:195:65], 1.0)

            # ---------- work generators ----------
            def gen_proj(n, borrow=False):
                """Projection for t-chunk n: 3 QK m-tiles + 4 V t-subchunks.
                borrow=True (prologue only): rotate the psum accumulator
                across pj AND the still-idle po banks (pohi/po2) to
                triple-buffer the early projection chain -- no attention
                unit touches those banks until round 1."""
                if n + 2 < NQ:
                    load_xn(n + 2)
                tctr = [0]

                def pj_tile(nm):
                    sel = tctr[0] % 3 if borrow else 0
                    tctr[0] += 1
                    if sel == 1:
                        return pshi.tile([128, 512], F32, tag='po2',
                                         name=nm, bufs=1)
                    if sel == 2:
                        return pshi.tile([128, 512], F32, tag='pohi',
                                         name=nm, bufs=1)
                    return pjp.tile([128, 512], F32, tag="pj", name=nm,
                                    bufs=1)

                for m in range(3):
                    pj = pj_tile(f"pj{n}_{m}")
                    for k in range(6):
                        nc.tensor.matmul(
                            pj[:],
                            lhsT=wqk_all[:, 384 * k + 128 * m:
                                         384 * k + 128 * (m + 1)],
                            rhs=xn[n][:, 512 * k:512 * (k + 1)],
                            start=(k == 0), stop=(k == 5))
                    yield
                    # psum->sbuf copy + per-partition bias on ACT
                    nc.scalar.activation(
                        out=mdst[m][n][:], in_=pj[:], func=AF.Identity,
                        bias=bias_qk[:, m:m + 1], scale=1.0)
                # duplicate Q/K with halves swapped so every head's Q/K is
                # available on BOTH PE row-halves (enables deterministic
                # intra-unit lo/hi matmul pairing); issued from the idle
                # GPSIMD queue to keep the sync queue free for I/O DMAs
                nc.gpsimd.dma_start(dup2[n][0:64, :], qk2[n][64:128, :])
                nc.gpsimd.dma_start(dup2[n][64:128, :], qk2[n][0:64, :])
                nc.gpsimd.dma_start(qs01[n][0:64, :], q01[n][64:128, :])
                nc.gpsimd.dma_start(qs01[n][64:128, :], q01[n][0:64, :])
                nc.gpsimd.dma_start(ks01[n][0:64, :], k01[n][64:128, :])
                nc.gpsimd.dma_start(ks01[n][64:128, :], k01[n][0:64, :])
                for ti in range(4):
                    kc = 4 * n + ti
                    vp = pj_tile(f"vp{kc}")
                    for k in range(6):
                        nc.tensor.matmul(
                            vp[:, 0:192],
                            lhsT=xn[n][:, 512 * k + 128 * ti:
                                       512 * k + 128 * (ti + 1)],
                            rhs=wv_all[:, 192 * k:192 * (k + 1)],
                            start=(k == 0), stop=(k == 5))
                    yield
                    # single strided copy psum->v_sb (V bias added on host)
                    dst = v_sb[:, VST * kc:VST * kc + 195].rearrange(
                        'p (h c) -> p h c', c=65)[:, :, 0:64]
                    nc.vector.tensor_copy(
                        dst, vp[:, 0:192].rearrange('p (h c) -> p h c', c=64))

            def unit(h, J):
                """One (head, q-chunk) attention unit. Each group's two S^T
                matmuls use Q/K operands on OPPOSITE PE row-halves and are
                emitted back-to-back with no yield between, so they share
                identical readiness and deterministically co-execute (~2x).
                They write different PSUM banks (one 512-col window each)."""
                def kq(side):
                    if h == 0:
                        return (k01, q01) if side == 'lo' else (ks01, qs01)
                    if h == 1:
                        return (ks01, qs01) if side == 'lo' else (k01, q01)
                    return (dup2, qk2) if side == 'lo' else (qk2, dup2)
                popool, potag = {0: (pslo, 'polo'), 1: (pshi, 'pohi'),
                                 2: (pshi, 'po2')}[h]
                po = popool.tile([65, 512], F32, tag=potag,
                                 name=f"po_{h}_{J}", bufs=1)
                # kc groups: full chunks over [0, 4J), then trimmed diagonal
                # blocks packed into 512-col PSUM-bank windows (a matmul
                # output must never cross a 2KB PSUM bank boundary)
                groups = []
                g0 = 0
                while g0 < 4 * J:
                    g1 = min(g0 + GRP, 4 * J)
                    groups.append(([(kc, 512 * (kc - g0), 512)
                                    for kc in range(g0, g1)], False))
                    g0 = g1
                blocks = [(4 * J + d, 512 - 128 * d) for d in range(4)]
                cur, fill = [], [0] * GRP
                for kc, w in blocks:
                    wi = next((i for i in range(GRP)
                               if 512 - fill[i] >= w), None)
                    if wi is None:
                        groups.append((cur, True))
                        cur, fill = [], [0] * GRP
                        wi = 0
                    cur.append((kc, 512 * wi + fill[wi], w))
                    fill[wi] += w
                groups.append((cur, True))
                pending = None

                def gen_pv(pend):
                    blks, diag, ppT, last = pend
                    for i, (kc, off, w) in enumerate(blks):
                        d = kc - 4 * J
                        qc0 = 128 * d if diag else 0
                        nc.tensor.matmul(
                            po[:, qc0:512],
                            lhsT=v_sb[:, VST * kc + 65 * h:
                                      VST * kc + 65 * h + 65],
                            rhs=ppT[:, off:off + w],
                            start=(kc == 0), stop=(last and i == len(blks) - 1),
                            skip_group_check=True)
                        yield

                for gi, (blks, diag) in enumerate(groups):
                    # ps/pT buffer sharing: h0 on ps_lo, h1 on ps_hi, h2
                    # alternates (WAR-serialized against h0/h1 alternately)
                    pspool, pst = ((pslo, 'pslo') if h == 0 else
                                   (pshi, 'pshi') if h == 1 else
                                   ((pslo, 'pslo') if gi % 2 == 0
                                    else (pshi, 'pshi')))
                    wid = max(off + w for _, off, w in blks)
                    ps = pspool.tile([128, 512 * GRP], F32, tag=pst,
                                     name=f"ps_{h}_{J}_{gi}", bufs=1)
                    for bi, (kc, off, w) in enumerate(blks):
                        # side by PSUM-bank window: co-executing pair members
                        # must write DIFFERENT banks; same-window blocks stay
                        # same-side so they serialize instead of colliding
                        side = 'lo' if (off // 512) % 2 == 0 else 'hi'
                        hof = 0 if side == 'lo' else 64
                        ktile, qtile = kq(side)
                        qc0 = 128 * (kc - 4 * J) if diag else 0
                        nc.tensor.matmul(
                            ps[:, off:off + w],
                            lhsT=ktile[kc // 4][hof:hof + 64,
                                               128 * (kc % 4):
                                               128 * (kc % 4 + 1)],
                            rhs=qtile[J][hof:hof + 64, qc0:512],
                            start=True, stop=True)
                    yield
                    pT = ptp.tile([128, 512 * GRP], BF, tag=pst,
                                  name=f"pT_{h}_{J}_{gi}", bufs=4)
                    # per-unit engine: h0 exps on ACT, h1 on DVE, h2
                    # alternates -- each unit's ps-WAR chain is paced by ONE
                    # engine queue, decoupling the units' stall phases
                    use_dve = (h == 1) if h != 2 else (gi % 2 == 1)
                    if use_dve:
                        nc.vector.tensor_scalar(
                            out=pT[:, :wid].bitcast(I16), in0=ps[:, :wid],
                            scalar1=EXP_A, scalar2=EXP_B,
                            op0=AL.mult, op1=AL.add)
                    else:
                        nc.scalar.activation(pT[:, :wid], ps[:, :wid],
                                             AF.Exp, scale=ACT_SCALE)
                    dve_ctr[0] += 1
                    if diag:
                        # zero leading [128,128] triangle (q_loc < k) of each
                        # diagonal block: bf16 mask multiply in DVE 2x mode
                        for kc, off, w in blks:
                            nc.vector.tensor_tensor(
                                out=pT[:, off:off + 128],
                                in0=pT[:, off:off + 128],
                                in1=mask_tri[:], op=AL.mult)
                    if pending is not None:
                        yield from gen_pv(pending)
                    pending = (blks, diag, pT, gi == len(groups) - 1)
                    yield
                yield from gen_pv(pending)
                yield
                po_sb = ptp.tile([65, 512], F32, tag=f"posb{h}",
                                 name=f"posb_{h}_{J}", bufs=3)
                # psum->sbuf staging on ACT (DVE queue is the busier one)
                nc.scalar.copy(po_sb[:], po[:])
                nc.sync.dma_start(
                    out_d[65 * h:65 * (h + 1), 512 * J:512 * (J + 1)], po_sb[:])

            # ---------- weave: proj(r) among attention units of J=r-1 ------
            def drive(gens, slow=(), cadence=3):
                # gens in `slow` advance every cadence-th cycle so their
                # filler work spreads across the whole round
                alive = [True] * len(gens)
                cyc = 0
                while any(alive):
                    for i, g in enumerate(gens):
                        if alive[i] and (i not in slow or cyc % cadence == 0
                                         or not any(alive[j] for j in
                                                    range(len(gens))
                                                    if j not in slow)):
                            try:
                                next(g)
                            except StopIteration:
                                alive[i] = False
                    cyc += 1

            # prologue: interleave proj(0) and proj(1) so proj(1)'s matmuls
            # fill proj(0)'s copy/DMA tail before round 1 starts
            drive([gen_proj(0, borrow=True), gen_proj(1, borrow=True)])
            for r in range(1, NQ + 1):
                J = r - 1
                gens = []
                if r + 1 < NQ:
                    # rounds 1-2 are still proj-chain-bound (tiny attention):
                    # keep borrowing the po banks, pre-advancing so the
                    # borrowed tiles allocate BEFORE the units' po tiles
                    pg = gen_proj(r + 1, borrow=(r <= 2))
                    if r <= 2:
                        next(pg)
                        next(pg)
                        next(pg)
                    gens.append(pg)
                u0, u1, u2 = unit(0, J), unit(1, J), unit(2, J)
                # de-phase the units so their exp-WAR stalls on the PE
                # FIFO don't synchronize: advance h1 by 1 and h2 by 2
                # yields before the round-robin weave starts
                next(u1)
                next(u1)
                next(u2)
                next(u2)
                next(u2)
                next(u2)
                gens += [u0, u1, u2]
                # spread the 7 proj bursts across the round
                cad = max(3, (14 * J + 20) // 8)
                drive(gens, slow=(0,) if r + 1 < NQ else (), cadence=cad)

    nc.compile()
    _CACHE['nc'] = nc
    return nc


def _prep_inputs(x, w_qkv, b_qkv):
    """Host-side sharding: per-core packed x + reordered weight stacks.

    xr[p, 3072n+512k+c] = x[b][512n+c, 128k+p]  (one contiguous DMA per n)
    wqk[p, 384k+j] = w_qk_stack[128k+p, j]; wv[p, 192k+j] = w_v_stack[...]
    """
    import ml_dtypes
    cdt = ml_dtypes.bfloat16
    f8dt = ml_dtypes.float8_e4m3fn
    x = np.asarray(x, dtype=np.float32)
    w_qkv = np.asarray(w_qkv, dtype=np.float32)
    b_qkv = np.asarray(b_qkv, dtype=np.float32)
    xrs = []
    for b in range(B):
        xT = x[b].T.astype(cdt)  # [C, T]
        xr = np.ascontiguousarray(
            xT.reshape(6, 128, NQ, 512).transpose(1, 2, 0, 3).reshape(
                128, NQ * 3072))
        xrs.append(xr)
    in_maps = []
    for c in range(NCORES):
        b_idx, g = c // 4, c % 4
        H = [3 * g, 3 * g + 1, 3 * g + 2]
        q = lambda h: np.arange(64 * h, 64 * (h + 1))
        k = lambda h: np.arange(C + 64 * h, C + 64 * (h + 1))
        v = lambda h: np.arange(2 * C + 64 * h, 2 * C + 64 * (h + 1))
        qk_cols = np.concatenate([
            q(H[0]), q(H[1]), k(H[0]), k(H[1]), q(H[2]), k(H[2])])
        v_cols = np.concatenate([v(H[0]), v(H[1]), v(H[2])])
        wqk = w_qkv[:, qk_cols].astype(cdt).reshape(
            6, 128, 384).transpose(1, 0, 2).reshape(128, 2304)
        wv = w_qkv[:, v_cols].astype(cdt).reshape(
            6, 128, 192).transpose(1, 0, 2).reshape(128, 1152)
        bqk = np.zeros((128, 3), dtype=np.float32)
        for m in range(3):
            bqk[:, m] = b_qkv[qk_cols[128 * m:128 * (m + 1)]]
        in_maps.append({"xr": xrs[b_idx],
                        "wqk": np.ascontiguousarray(wqk),
                        "wv": np.ascontiguousarray(wv),
                        "bqk": bqk})
    return in_maps


def _run(x, w_qkv, b_qkv, n_head, **run_kwargs):
    assert int(n_head) == NH and x.shape == (B, T, C)
    nc = _build()
    in_maps = _prep_inputs(x, w_qkv, b_qkv)
    res = bass_utils.run_bass_kernel_spmd(
        nc, in_maps, core_ids=list(range(NCORES)), **run_kwargs)
    b_qkv = np.asarray(b_qkv, dtype=np.float32)
    out = np.empty((B, T, C), dtype=np.float32)
    for c in range(NCORES):
        b_idx, g = c // 4, c % 4
        o = res.results[c]["out"]  # [195, T]
        for h in range(HPC):
            ot = o[65 * h:65 * h + 64, :]       # unnormalized O^T
            den = o[65 * h + 64:65 * h + 65, :]  # softmax denominator
            bv = b_qkv[2 * C + 192 * g + 64 * h:2 * C + 192 * g + 64 * (h + 1)]
            out[b_idx, :, 192 * g + 64 * h:192 * g + 64 * (h + 1)] = (
                (ot / den).T + bv[None, :])
    return out, res


def kernel(x, w_qkv, b_qkv, n_head):
    return _run(x, w_qkv, b_qkv, n_head)[0]


# revision 52
# speedup vs baseline: 1.0278x; 1.0060x over previous
"""Trainium2 Bass kernel: fused causal MHA (qkv proj + causal softmax attn),
B=2, T=4096, C=768, nH=12, hd=64.

Sharding: 8 cores; core c -> batch b=c//4, head group g=c%4 (3 heads/core).

Per-core design (all matmuls bf16 operands, fp32 PSUM):
  Q^T/K^T proj: full cd=128 matmuls -> m-tiles [Q0;Q1],[K0;K1],[Q2;K2].
  V proj DIRECTLY in [key,dim] layout: out[t,d] = xT_chunk^T @ w_v chunk
    (lhsT = xT chunk, rhs = w_v) -- no PE transposes needed.
  S^T[k,q] = K Q^T as cd=64 matmuls on PE row-half tiles (lo=rows 0:63,
    hi=64:127). MMs on disjoint row groups CO-EXECUTE only when adjacent
    in program order, so the three per-round units are driven per-MM:
    h0 always lo, h1 always hi, h2 alternates sides per exp-group (its
    Q/K live in both halves via the swapped dup2 tile) -> each side gets
    ~1.5 units/round and S^T runs at ~2x column rate.
  Causal trim: for q-chunk J, diagonal kc blocks use rhs width 512-128d;
    leading [128,128] triangle of exp'd P^T zeroed by bf16 mask multiply.
  exp split across two engines: ACT (table exp, scale=1/8) and DVE
    (Schraudolph: pT_bf16 = bitcast(int16(A*S + B)), one tensor_scalar
    mult+add pass with f32->i16 round-to-nearest convert; ~3% per-element
    which the softmax normalization mostly averages out).
  PV: O^T_aug[65,512] += V_aug^T P^T (cd=128, ones col gives denom row).
  Output: unnormalized [O^T;denom] rows psum->sbuf->DRAM; HOST divides by
    denominator, transposes, and adds the V bias (out = num/den + b_v).
  Q/K bias: ACT Identity-with-bias on the psum->sbuf copy (per-partition).
  Schedule: proj chunk n is woven between attention units of round n-1.
PSUM budget: ps_lo 2 + po_h0 1 + ps_hi 2 + po_h1 1 + po_h2 1 + pj 1 = 8.
"""
import sys
sys.path.insert(0, '/opt/trn_rl_repo')
import numpy as np

import concourse.bass as bass
import concourse.tile as tile
from concourse import bacc, mybir
from concourse import bass_utils

B, T, C, NH = 2, 4096, 768, 12
HD = 64
HPC = 3
NCORES = 8
NQ = T // 512   # 8 q-chunks of 512
NKC = T // 128  # 32 key blocks of 128
VST = 208       # v_sb per-kc block stride (3*65=195 used, pad to 208)
GRP = 2         # S^T psum-bank group width (both sides)

# Schraudolph exp: bf16 = bitcast(int16(round(EXP_A*S + EXP_B)))
EXP_A = 184.6650558 * 0.125
EXP_B = 16248.6
ACT_SCALE = 0.125
# exp-group engine assignment: group ctr % DVE_PERIOD in DVE_SLOTS -> DVE
DVE_PERIOD = 7
DVE_SLOTS = (0, 2, 4)

BF = mybir.dt.bfloat16
F32 = mybir.dt.float32
F8 = mybir.dt.float8e4
I16 = mybir.dt.int16
AF = mybir.ActivationFunctionType
AL = mybir.AluOpType
DR = mybir.MatmulPerfMode.DoubleRow

_CACHE = {}


def _build():
    if 'nc' in _CACHE:
        return _CACHE['nc']
    nc = bacc.Bacc("TRN2", target_bir_lowering=False, debug=False,
                   enable_asserts=True, num_devices=NCORES)
    # host-packed layouts: one contiguous DMA per load
    #   xr[p, 3072n+512k+c] = x[b][512n+c, 128k+p]
    #   wqk[p, 384k+j]      = w_qk_stack[128k+p, j]   (j = 128m+c2)
    #   wv[p, 192k+j]       = w_v_stack[128k+p, j]
    xr_d = nc.dram_tensor("xr", [128, NQ * 3072], BF, kind="ExternalInput").ap()
    wqk_d = nc.dram_tensor("wqk", [128, 2304], BF, kind="ExternalInput").ap()
    wv_d = nc.dram_tensor("wv", [128, 1152], BF, kind="ExternalInput").ap()
    bqk_d = nc.dram_tensor("bqk", [128, 3], F32, kind="ExternalInput").ap()
    out_d = nc.dram_tensor("out", [HPC * 65, T], F32, kind="ExternalOutput").ap()

    dve_ctr = [0]

    with tile.TileContext(nc) as tc:
        with (
            tc.tile_pool(name="const", bufs=1) as cpool,
            tc.tile_pool(name="persist", bufs=1) as sb,
            tc.tile_pool(name="xn", bufs=4) as xpool,
            tc.tile_pool(name="pT", bufs=1) as ptp,
            tc.tile_pool(name="pj", bufs=1, space="PSUM") as pjp,
            tc.tile_pool(name="ps_lo", bufs=1, space="PSUM") as pslo,
            tc.tile_pool(name="ps_hi", bufs=1, space="PSUM") as pshi,
        ):
            # ---------- input DMAs first (hide latency) ----------
            xn = {}  # n -> [128, 3072] tile, chunk k at cols 512k:512(k+1)

            def load_xn(n):
                t = xpool.tile([128, 3072], BF, tag="xn", name=f"xn{n}")
                nc.sync.dma_start(t[:], xr_d[:, 3072 * n:3072 * (n + 1)])
                xn[n] = t

            t0 = xpool.tile([128, 3072], BF, tag="xn", name="xn0")
            wqk_all = sb.tile([128, 2304], BF, name="wqk")
            # tiny bias DMA first: the first Identity copy must not wait
            # behind the 12 big chunk DMAs in the queue
            bias_qk = cpool.tile([128, 3], F32)
            nc.sync.dma_start(bias_qk[:], bqk_d[:])
            for k in range(6):
                nc.sync.dma_start(t0[:, 512 * k:512 * (k + 1)],
                                  xr_d[:, 512 * k:512 * (k + 1)],
                                  single_packet=True)
                nc.gpsimd.dma_start(wqk_all[:, 384 * k:384 * (k + 1)],
                                    wqk_d[:, 384 * k:384 * (k + 1)],
                                    single_packet=True)
            xn[0] = t0
            wv_all = sb.tile([128, 1152], BF, name="wv")
            nc.sync.dma_start(wv_all[:], wv_d[:])
            load_xn(1)

            # ---------- constants ----------
            mask_tri = cpool.tile([128, 128], BF)
            nc.gpsimd.memset(mask_tri[:], 1.0)
            nc.gpsimd.affine_select(
                out=mask_tri[:], in_=mask_tri[:], compare_op=AL.is_ge,
                fill=0.0, base=0, channel_multiplier=-1, pattern=[[1, 128]])

            # Q/K tiles per n: q01=[Q0;Q1] k01=[K0;K1] qk2=[Q2;K2]
            # dup2=[K2;Q2] (halves swapped, via sbuf DMA)
            q01 = [sb.tile([128, 512], BF, name=f"q01_{n}") for n in range(NQ)]
            k01 = [sb.tile([128, 512], BF, name=f"k01_{n}") for n in range(NQ)]
            qk2 = [sb.tile([128, 512], BF, name=f"qk2_{n}") for n in range(NQ)]
            dup2 = [sb.tile([128, 512], BF, name=f"dup2_{n}") for n in range(NQ)]
            qs01 = [sb.tile([128, 512], BF, name=f"qs01_{n}") for n in range(NQ)]
            ks01 = [sb.tile([128, 512], BF, name=f"ks01_{n}") for n in range(NQ)]
            mdst = [q01, k01, qk2]
            # V storage: per kc block of 128 keys: [65 h0][65 h1][65 h2][pad]
            # with col 65h+64 = 1.0 (softmax denominator via ones column).
            v_sb = sb.tile([128, NKC * VST], BF, name="v_sb")
            # only the ones-columns need init (data cols written by proj,
            # pad cols never read): strided memset over cols VST*kc+65h+64
            v_r = v_sb[:].rearrange('p (a b) -> p a b', b=VST)
            nc.vector.memset(v_r[:, :, 64:195:65], 1.0)

            # ---------- work generators ----------
            def gen_proj(n, borrow=False):
                """Projection for t-chunk n: 3 QK m-tiles + 4 V t-subchunks.
                borrow=True (prologue only): rotate the psum accumulator
                across pj AND the still-idle po banks (pohi/po2) to
                triple-buffer the early projection chain -- no attention
                unit touches those banks until round 1."""
                if n + 2 < NQ:
                    load_xn(n + 2)
                tctr = [0]

                def pj_tile(nm):
                    sel = tctr[0] % 3 if borrow else 0
                    tctr[0] += 1
                    if sel == 1:
                        return pshi.tile([128, 512], F32, tag='po2',
                                         name=nm, bufs=1)
                    if sel == 2:
                        return pshi.tile([128, 512], F32, tag='pohi',
                                         name=nm, bufs=1)
                    return pjp.tile([128, 512], F32, tag="pj", name=nm,
                                    bufs=1)

                for m in range(3):
                    pj = pj_tile(f"pj{n}_{m}")
                    for k in range(6):
                        nc.tensor.matmul(
                            pj[:],
                            lhsT=wqk_all[:, 384 * k + 128 * m:
                                         384 * k + 128 * (m + 1)],
                            rhs=xn[n][:, 512 * k:512 * (k + 1)],
                            start=(k == 0), stop=(k == 5))
                    yield
                    # psum->sbuf copy + per-partition bias on ACT
                    nc.scalar.activation(
                        out=mdst[m][n][:], in_=pj[:], func=AF.Identity,
                        bias=bias_qk[:, m:m + 1], scale=1.0)
                # duplicate Q/K with halves swapped so every head's Q/K is
                # available on BOTH PE row-halves (enables deterministic
                # intra-unit lo/hi matmul pairing); issued from the idle
                # GPSIMD queue to keep the sync queue free for I/O DMAs
                nc.gpsimd.dma_start(dup2[n][0:64, :], qk2[n][64:128, :])
                nc.gpsimd.dma_start(dup2[n][64:128, :], qk2[n][0:64, :])
                nc.gpsimd.dma_start(qs01[n][0:64, :], q01[n][64:128, :])
                nc.gpsimd.dma_start(qs01[n][64:128, :], q01[n][0:64, :])
                nc.gpsimd.dma_start(ks01[n][0:64, :], k01[n][64:128, :])
                nc.gpsimd.dma_start(ks01[n][64:128, :], k01[n][0:64, :])
                for ti in range(4):
                    kc = 4 * n + ti
                    vp = pj_tile(f"vp{kc}")
                    for k in range(6):
                        nc.tensor.matmul(
                            vp[:, 0:192],
                            lhsT=xn[n][:, 512 * k + 128 * ti:
                                       512 * k + 128 * (ti + 1)],
                            rhs=wv_all[:, 192 * k:192 * (k + 1)],
                            start=(k == 0), stop=(k == 5))
                    yield
                    # single strided copy psum->v_sb (V bias added on host)
                    dst = v_sb[:, VST * kc:VST * kc + 195].rearrange(
                        'p (h c) -> p h c', c=65)[:, :, 0:64]
                    nc.vector.tensor_copy(
                        dst, vp[:, 0:192].rearrange('p (h c) -> p h c', c=64))

            def unit(h, J):
                """One (head, q-chunk) attention unit. Each group's two S^T
                matmuls use Q/K operands on OPPOSITE PE row-halves and are
                emitted back-to-back with no yield between, so they share
                identical readiness and deterministically co-execute (~2x).
                They write different PSUM banks (one 512-col window each)."""
                def kq(side):
                    if h == 0:
                        return (k01, q01) if side == 'lo' else (ks01, qs01)
                    if h == 1:
                        return (ks01, qs01) if side == 'lo' else (k01, q01)
                    return (dup2, qk2) if side == 'lo' else (qk2, dup2)
                popool, potag = {0: (pslo, 'polo'), 1: (pshi, 'pohi'),
                                 2: (pshi, 'po2')}[h]
                po = popool.tile([65, 512], F32, tag=potag,
                                 name=f"po_{h}_{J}", bufs=1)
                # kc groups: full chunks over [0, 4J), then trimmed diagonal
                # blocks packed into 512-col PSUM-bank windows (a matmul
                # output must never cross a 2KB PSUM bank boundary)
                groups = []
                g0 = 0
                while g0 < 4 * J:
                    g1 = min(g0 + GRP, 4 * J)
                    groups.append(([(kc, 512 * (kc - g0), 512)
                                    for kc in range(g0, g1)], False))
                    g0 = g1
                blocks = [(4 * J + d, 512 - 128 * d) for d in range(4)]
                cur, fill = [], [0] * GRP
                for kc, w in blocks:
                    wi = next((i for i in range(GRP)
                               if 512 - fill[i] >= w), None)
                    if wi is None:
                        groups.append((cur, True))
                        cur, fill = [], [0] * GRP
                        wi = 0
                    cur.append((kc, 512 * wi + fill[wi], w))
                    fill[wi] += w
                groups.append((cur, True))
                pending = None

                def gen_pv(pend):
                    blks, diag, ppT, last = pend
                    for i, (kc, off, w) in enumerate(blks):
                        d = kc - 4 * J
                        qc0 = 128 * d if diag else 0
                        nc.tensor.matmul(
                            po[:, qc0:512],
                            lhsT=v_sb[:, VST * kc + 65 * h:
                                      VST * kc + 65 * h + 65],
                            rhs=ppT[:, off:off + w],
                            start=(kc == 0), stop=(last and i == len(blks) - 1),
                            skip_group_check=True)
                        yield

                for gi, (blks, diag) in enumerate(groups):
                    # ps/pT buffer sharing: h0 on ps_lo, h1 on ps_hi, h2
                    # alternates (WAR-serialized against h0/h1 alternately)
                    pspool, pst = ((pslo, 'pslo') if h == 0 else
                                   (pshi, 'pshi') if h == 1 else
                                   ((pslo, 'pslo') if gi % 2 == 0
                                    else (pshi, 'pshi')))
                    wid = max(off + w for _, off, w in blks)
                    ps = pspool.tile([128, 512 * GRP], F32, tag=pst,
                                     name=f"ps_{h}_{J}_{gi}", bufs=1)
                    for bi, (kc, off, w) in enumerate(blks):
                        # side by PSUM-bank window: co-executing pair members
                        # must write DIFFERENT banks; same-window blocks stay
                        # same-side so they serialize instead of colliding
                        side = 'lo' if (off // 512) % 2 == 0 else 'hi'
                        hof = 0 if side == 'lo' else 64
                        ktile, qtile = kq(side)
                        qc0 = 128 * (kc - 4 * J) if diag else 0
                        nc.tensor.matmul(
                            ps[:, off:off + w],
                            lhsT=ktile[kc // 4][hof:hof + 64,
                                               128 * (kc % 4):
                                               128 * (kc % 4 + 1)],
                            rhs=qtile[J][hof:hof + 64, qc0:512],
                            start=True, stop=True)
                    yield
                    pT = ptp.tile([128, 512 * GRP], BF, tag=pst,
                                  name=f"pT_{h}_{J}_{gi}", bufs=4)
                    # per-unit engine: h0 exps on ACT, h1 on DVE, h2
                    # alternates -- each unit's ps-WAR chain is paced by ONE
                    # engine queue, decoupling the units' stall phases
                    use_dve = (h == 1) if h != 2 else (gi % 2 == 1)
                    if use_dve:
                        nc.vector.tensor_scalar(
                            out=pT[:, :wid].bitcast(I16), in0=ps[:, :wid],
                            scalar1=EXP_A, scalar2=EXP_B,
                            op0=AL.mult, op1=AL.add)
                    else:
                        nc.scalar.activation(pT[:, :wid], ps[:, :wid],
                                             AF.Exp, scale=ACT_SCALE)
                    dve_ctr[0] += 1
                    if diag:
                        # zero leading [128,128] triangle (q_loc < k) of each
                        # diagonal block: bf16 mask multiply in DVE 2x mode
                        for kc, off, w in blks:
                            nc.vector.tensor_tensor(
                                out=pT[:, off:off + 128],
                                in0=pT[:, off:off + 128],
                                in1=mask_tri[:], op=AL.mult)
                    if pending is not None:
                        yield from gen_pv(pending)
                    pending = (blks, diag, pT, gi == len(groups) - 1)
                    yield
                yield from gen_pv(pending)
                yield
                po_sb = ptp.tile([65, 512], F32, tag=f"posb{h}",
                                 name=f"posb_{h}_{J}", bufs=3)
                # psum->sbuf staging on ACT (DVE queue is the busier one)
                nc.scalar.copy(po_sb[:], po[:])
                nc.sync.dma_start(
                    out_d[65 * h:65 * (h + 1), 512 * J:512 * (J + 1)], po_sb[:])

            # ---------- weave: proj(r) among attention units of J=r-1 ------
            def drive(gens, slow=(), cadence=3):
                # gens in `slow` advance every cadence-th cycle so their
                # filler work spreads across the whole round
                alive = [True] * len(gens)
                cyc = 0
                while any(alive):
                    for i, g in enumerate(gens):
                        if alive[i] and (i not in slow or cyc % cadence == 0
                                         or not any(alive[j] for j in
                                                    range(len(gens))
                                                    if j not in slow)):
                            try:
                                next(g)
                            except StopIteration:
                                alive[i] = False
                    cyc += 1

            # prologue: interleave proj(0) and proj(1) so proj(1)'s matmuls
            # fill proj(0)'s copy/DMA tail before round 1 starts
            drive([gen_proj(0, borrow=True), gen_proj(1, borrow=True)])
            for r in range(1, NQ + 1):
                J = r - 1
                gens = []
                if r + 1 < NQ:
                    # rounds 1-2 are still proj-chain-bound (tiny attention):
                    # keep borrowing the po banks, pre-advancing so the
                    # borrowed tiles allocate BEFORE the units' po tiles
                    pg = gen_proj(r + 1, borrow=(r <= 3))
                    if r <= 3:
                        next(pg)
                        next(pg)
                        next(pg)
                    gens.append(pg)
                u0, u1, u2 = unit(0, J), unit(1, J), unit(2, J)
                # de-phase the units so their exp-WAR stalls on the PE
                # FIFO don't synchronize: advance h1 by 1 and h2 by 2
                # yields before the round-robin weave starts
                next(u1)
                next(u1)
                next(u2)
                next(u2)
                next(u2)
                next(u2)
                gens += [u0, u1, u2]
                # spread the 7 proj bursts across the round
                cad = max(3, (14 * J + 20) // 8)
                drive(gens, slow=(0,) if r + 1 < NQ else (), cadence=cad)

    nc.compile()
    _CACHE['nc'] = nc
    return nc


def _prep_inputs(x, w_qkv, b_qkv):
    """Host-side sharding: per-core packed x + reordered weight stacks.

    xr[p, 3072n+512k+c] = x[b][512n+c, 128k+p]  (one contiguous DMA per n)
    wqk[p, 384k+j] = w_qk_stack[128k+p, j]; wv[p, 192k+j] = w_v_stack[...]
    """
    import ml_dtypes
    cdt = ml_dtypes.bfloat16
    f8dt = ml_dtypes.float8_e4m3fn
    x = np.asarray(x, dtype=np.float32)
    w_qkv = np.asarray(w_qkv, dtype=np.float32)
    b_qkv = np.asarray(b_qkv, dtype=np.float32)
    xrs = []
    for b in range(B):
        xT = x[b].T.astype(cdt)  # [C, T]
        xr = np.ascontiguousarray(
            xT.reshape(6, 128, NQ, 512).transpose(1, 2, 0, 3).reshape(
                128, NQ * 3072))
        xrs.append(xr)
    in_maps = []
    for c in range(NCORES):
        b_idx, g = c // 4, c % 4
        H = [3 * g, 3 * g + 1, 3 * g + 2]
        q = lambda h: np.arange(64 * h, 64 * (h + 1))
        k = lambda h: np.arange(C + 64 * h, C + 64 * (h + 1))
        v = lambda h: np.arange(2 * C + 64 * h, 2 * C + 64 * (h + 1))
        qk_cols = np.concatenate([
            q(H[0]), q(H[1]), k(H[0]), k(H[1]), q(H[2]), k(H[2])])
        v_cols = np.concatenate([v(H[0]), v(H[1]), v(H[2])])
        wqk = w_qkv[:, qk_cols].astype(cdt).reshape(
            6, 128, 384).transpose(1, 0, 2).reshape(128, 2304)
        wv = w_qkv[:, v_cols].astype(cdt).reshape(
            6, 128, 192).transpose(1, 0, 2).reshape(128, 1152)
        bqk = np.zeros((128, 3), dtype=np.float32)
        for m in range(3):
            bqk[:, m] = b_qkv[qk_cols[128 * m:128 * (m + 1)]]
        in_maps.append({"xr": xrs[b_idx],
                        "wqk": np.ascontiguousarray(wqk),
                        "wv": np.ascontiguousarray(wv),
                        "bqk": bqk})
    return in_maps


def _run(x, w_qkv, b_qkv, n_head, **run_kwargs):
    assert int(n_head) == NH and x.shape == (B, T, C)
    nc = _build()
    in_maps = _prep_inputs(x, w_qkv, b_qkv)
    res = bass_utils.run_bass_kernel_spmd(
        nc, in_maps, core_ids=list(range(NCORES)), **run_kwargs)
    b_qkv = np.asarray(b_qkv, dtype=np.float32)
    out = np.empty((B, T, C), dtype=np.float32)
    for c in range(NCORES):
        b_idx, g = c // 4, c % 4
        o = res.results[c]["out"]  # [195, T]
        for h in range(HPC):
            ot = o[65 * h:65 * h + 64, :]       # unnormalized O^T
            den = o[65 * h + 64:65 * h + 65, :]  # softmax denominator
            bv = b_qkv[2 * C + 192 * g + 64 * h:2 * C + 192 * g + 64 * (h + 1)]
            out[b_idx, :, 192 * g + 64 * h:192 * g + 64 * (h + 1)] = (
                (ot / den).T + bv[None, :])
    return out, res


def kernel(x, w_qkv, b_qkv, n_head):
    return _run(x, w_qkv, b_qkv, n_head)[0]
